# revision 4
# baseline (speedup 1.0000x reference)
"""DGCNN (4x EdgeConv) Trainium2 Bass kernel.

Sharding: data-parallel over batch B=8 across 8 NeuronCores (one point cloud
per core). BN batch statistics are combined with a tiny (5*O floats) AllReduce
per block.

Key algebraic restructuring: with W = [Wn | Wc] the edge conv
    h[n,kk,:] = (x[idx[n,kk]] - x[n]) @ Wn.T + x[n] @ Wc.T
              = U[idx[n,kk]] + V[n],   U = X @ Wn.T,  V = X @ (Wc-Wn).T
so the per-edge matmul (N*k rows) becomes two per-point matmuls (20x fewer
FLOPs) followed by a row gather of U (dma_gather). Since BN scale s>0 and
LeakyReLU is monotone increasing, max over k commutes with the activation:
    out[n] = lrelu(s * (max_k U[idx[n,k]] + V[n]) + t)
BN statistics over h are recovered exactly from reductions:
    sum(h)   = sum(sumU) + k*sum(V)
    sum(h^2) = sum_j cnt_j U_j^2 + 2*sum(V*sumU) + k*sum(V^2)
where cnt_j is the in-degree of point j (column sums of the kNN selection
mask, done on the tensor engine) and sumU[n] = sum_k U[idx[n,k]].

kNN: E = 2*X^T X - sq_j (row-shifted negative squared distance; the -sq_i
term is constant per row and irrelevant for row-wise top-k) via an augmented
fp32 matmul; top-20 per row via 3 rounds of max8/max_index/match_replace.
"""

import numpy as np

N = 2048
KNN = 20
NCORES = 8
P = 128
BLOCKS = [(3, 64), (64, 64), (64, 128), (128, 256)]  # (C_in, C_out)
OUT_F = sum(o for _, o in BLOCKS)  # 512
EPS = 1e-5
NEG = -1e30


def build_nc(num_cores=NCORES, n=N):
    import concourse.bacc as bacc
    import concourse.mybir as mybir
    import concourse.tile as tile
    from concourse import library_config

    f32 = mybir.dt.float32
    bf16 = mybir.dt.bfloat16
    T = n // P          # point tiles per cloud
    CH = n // 512       # 512-wide chunks per row
    M_total = num_cores * n * KNN

    nc = bacc.Bacc("TRN2", target_bir_lowering=False, debug=False,
                   num_devices=num_cores)

    xt_in = nc.dram_tensor("xt_in", [3, n], f32, kind="ExternalInput")
    ident_in = nc.dram_tensor("ident_in", [P, P], f32, kind="ExternalInput")
    wn_in, wd_in, g_in, b_in = {}, {}, {}, {}
    for i, (c, o) in enumerate(BLOCKS):
        wn_in[i] = nc.dram_tensor(f"wn{i}", [c, o], f32, kind="ExternalInput")
        wd_in[i] = nc.dram_tensor(f"wd{i}", [c, o], f32, kind="ExternalInput")
        g_in[i] = nc.dram_tensor(f"g{i}", [1, o], f32, kind="ExternalInput")
        b_in[i] = nc.dram_tensor(f"b{i}", [1, o], f32, kind="ExternalInput")
    out_dram = nc.dram_tensor("out", [n, OUT_F], f32, kind="ExternalOutput")

    OMAX = max(o for _, o in BLOCKS)

    with tile.TileContext(nc) as tc:
        with (
            tc.tile_pool(name="consts", bufs=1) as cp,
            tc.tile_pool(name="sb", bufs=1) as sb,
            tc.tile_pool(name="ps", bufs=2, space="PSUM") as ps,
            tc.tile_pool(name="dr", bufs=1, space="DRAM") as dr,
        ):
            nc.gpsimd.load_library(library_config.mlp)

            ident = cp.tile([P, P], f32)
            nc.sync.dma_start(out=ident, in_=ident_in.ap())
            ones_f = cp.tile([P, 1], f32)
            nc.vector.memset(ones_f, 1.0)
            ones_bf = cp.tile([P, 1], bf16)
            nc.vector.memset(ones_bf, 1.0)
            negones = cp.tile([1, P], f32)
            nc.vector.memset(negones, -1.0)
            wn_sb, wd_sb, g_sb, b_sb = {}, {}, {}, {}
            for i, (c, o) in enumerate(BLOCKS):
                wn_sb[i] = cp.tile([c, o], f32, name=f"wn_sb{i}")
                nc.sync.dma_start(out=wn_sb[i], in_=wn_in[i].ap())
                wd_sb[i] = cp.tile([c, o], f32, name=f"wd_sb{i}")
                nc.sync.dma_start(out=wd_sb[i], in_=wd_in[i].ap())
                g_sb[i] = cp.tile([1, o], f32, name=f"g_sb{i}")
                nc.sync.dma_start(out=g_sb[i], in_=g_in[i].ap())
                b_sb[i] = cp.tile([1, o], f32, name=f"b_sb{i}")
                nc.sync.dma_start(out=b_sb[i], in_=b_in[i].ap())

            xcm_tiles = [sb.tile([P, n], f32, name=f"xcm{j}", tag=f"xcm{j}")
                         for j in range(2)]
            nc.sync.dma_start(out=xcm_tiles[0][0:3, :], in_=xt_in.ap())

            for i, (C, O) in enumerate(BLOCKS):
                xcm = xcm_tiles[i % 2][0:C, :]
                xcm_next = xcm_tiles[(i + 1) % 2]
                wn, wd = wn_sb[i], wd_sb[i]
                OH = min(O, P)          # channel half for gather of O=256
                NH = 2 if O > P else 1

                # ---- prep: sq, 2x, U, V ----
                xtmp = sb.tile([P, n], f32, tag="xtmp", name=f"xtmp{i}")
                nc.scalar.square(out=xtmp[0:C, :], in_=xcm)
                sq_row = sb.tile([1, n], f32, tag="sq", name=f"sqr{i}")
                for ci in range(CH):
                    sl = slice(ci * 512, (ci + 1) * 512)
                    sq_ps = ps.tile([1, 512], f32, tag="aux", name=f"sqps{i}_{ci}")
                    nc.tensor.matmul(out=sq_ps, lhsT=ones_f[0:C, :],
                                     rhs=xtmp[0:C, sl], start=True, stop=True)
                    nc.scalar.copy(out=sq_row[:, sl], in_=sq_ps)
                xcm2 = sb.tile([P, n], f32, tag="xcm2", name=f"xcm2_{i}")
                nc.vector.tensor_scalar_mul(xcm2[0:C, :], xcm, 2.0)

                urm = sb.tile([P, T, O], f32, tag="urm", name=f"urm{i}")
                vrm = sb.tile([P, T, O], f32, tag="vrm", name=f"vrm{i}")
                for m in range(T):
                    msl = slice(m * P, (m + 1) * P)
                    for di, (dst, w_) in enumerate(((urm, wn), (vrm, wd))):
                        ups = ps.tile([P, O], f32, tag="aux",
                                      name=f"uv{i}_{m}_{di}")
                        nc.tensor.matmul(out=ups, lhsT=xcm[:, msl], rhs=w_,
                                         start=True, stop=True)
                        nc.scalar.copy(out=dst[:, m, :], in_=ups)
                u_dram = dr.tile([n, O], f32, tag="udram", name=f"ud{i}")
                nc.sync.dma_start(
                    out=u_dram.rearrange("(t p) o -> p t o", p=P), in_=urm)

                # ---- kNN + gather + windowed reduces ----
                idx_all = sb.tile([P, T, KNN], mybir.dt.int16, tag="idxall",
                                  name=f"idxall{i}")
                maxu = sb.tile([P, T, O], f32, tag="maxu", name=f"maxu{i}")
                sumu = sb.tile([P, T, O], f32, tag="sumu", name=f"sumu{i}")
                cnt_ps = [ps.tile([1, 512], f32, tag=f"cnt{ci}", bufs=1,
                                  name=f"cnt{i}_{ci}") for ci in range(CH)]

                def do_gather_tile(m, w_grp, ti, i=i, u_dram=u_dram, maxu=maxu,
                                   sumu=sumu, O=O, OH=OH, NH=NH):
                    for h in range(NH):
                        g_t = sb.tile([P, KNN, OH], f32, tag="gt", bufs=2,
                                      name=f"g{i}_{m}_{h}")
                        nc.gpsimd.dma_gather(
                            out_ap=g_t,
                            in_ap=u_dram[:, h * OH:(h + 1) * OH],
                            idxs_ap=w_grp[:, ti * 160:(ti + 1) * 160],
                            num_idxs=P * KNN, num_idxs_reg=P * KNN,
                            elem_size=OH,
                            elem_step=O if NH > 1 else None,
                            single_packet=False)
                        gv = g_t.rearrange("p k o -> p o k")
                        osl = slice(h * OH, (h + 1) * OH)
                        nc.vector.tensor_reduce(
                            out=maxu[:, m, osl], in_=gv,
                            axis=mybir.AxisListType.X, op=mybir.AluOpType.max)
                        nc.vector.tensor_reduce(
                            out=sumu[:, m, osl], in_=gv,
                            axis=mybir.AxisListType.X, op=mybir.AluOpType.add)

                GRP = 4
                for m in range(T):
                    msl = slice(m * P, (m + 1) * P)
                    e_m = sb.tile([P, n], f32, tag="em", bufs=2, name=f"e{i}_{m}")
                    for ci in range(CH):
                        sl = slice(ci * 512, (ci + 1) * 512)
                        eps_t = ps.tile([P, 512], f32, tag="eps",
                                        name=f"eps{i}_{m}_{ci}")
                        nc.tensor.matmul(out=eps_t, lhsT=xcm2[0:C, msl],
                                         rhs=xcm[:, sl], start=True, stop=False,
                                         skip_group_check=True)
                        nc.tensor.matmul(out=eps_t, lhsT=negones,
                                         rhs=sq_row[:, sl], start=False,
                                         stop=True, skip_group_check=True)
                        nc.scalar.copy(out=e_m[:, sl], in_=eps_t)

                    vals = sb.tile([P, 24], f32, tag="vals", bufs=2,
                                   name=f"v{i}_{m}")
                    idxu = sb.tile([P, 24], mybir.dt.uint32, tag="idxu", bufs=2,
                                   name=f"ixu{i}_{m}")
                    e_w = sb.tile([P, n], f32, tag="ew", bufs=1, name=f"ew{i}_{m}")
                    for r in range(3):
                        src = e_m if r == 0 else e_w
                        v8 = vals[:, r * 8:(r + 1) * 8]
                        nc.vector.max(out=v8, in_=src)
                        nc.vector.max_index(out=idxu[:, r * 8:(r + 1) * 8],
                                            in_max=v8, in_values=e_m)
                        if r < 2:
                            nc.vector.match_replace(out=e_w, in_to_replace=v8,
                                                    in_values=src, imm_value=NEG)
                    # selection mask + in-degree partial sums (PE column sums)
                    s_m = sb.tile([P, n], bf16, tag="sm", bufs=2, name=f"s{i}_{m}")
                    nc.vector.tensor_scalar(out=s_m, in0=e_m,
                                            scalar1=vals[:, 19:20], scalar2=None,
                                            op0=mybir.AluOpType.is_ge)
                    for ci in range(CH):
                        sl = slice(ci * 512, (ci + 1) * 512)
                        nc.tensor.matmul(out=cnt_ps[ci], lhsT=ones_bf,
                                         rhs=s_m[:, sl], start=(m == 0),
                                         stop=(m == T - 1), skip_group_check=True)
                    nc.vector.tensor_copy(idx_all[:, m, :], idxu[:, 0:KNN])

                    if m % GRP == GRP - 1:
                        g0 = m - (GRP - 1)
                        w_grp = sb.tile([P, GRP * 160], mybir.dt.int16,
                                        tag="wgrp", bufs=2, name=f"w{i}_{m}")
                        wv = w_grp.rearrange("l (t c pg) -> l t c pg", t=GRP, pg=8)
                        for pg in range(8):
                            nc.sync.dma_start(
                                out=wv[0:16, :, :, pg],
                                in_=idx_all[pg * 16:(pg + 1) * 16,
                                            g0:g0 + GRP, 0:KNN])
                        for rep in range(1, 8):
                            nc.scalar.dma_start(
                                out=w_grp[rep * 16:(rep + 1) * 16, :],
                                in_=w_grp[0:16, :])
                        for ti in range(GRP):
                            do_gather_tile(g0 + ti, w_grp, ti)

                # ---- BN statistics (per-channel column sums on PE) ----
                stats = sb.tile([1, 5 * O], f32, tag="stats", name=f"stt{i}")

                def col_sum(dst_sl, rhs_fn, lhs=None, i=i, stats=stats, O=O):
                    p_ps = ps.tile([1, O], f32, tag="aux",
                                   name=f"pps{i}_{dst_sl.start}")
                    for m in range(T):
                        nc.tensor.matmul(out=p_ps,
                                         lhsT=(ones_f if lhs is None else lhs(m)),
                                         rhs=rhs_fn(m), start=(m == 0),
                                         stop=(m == T - 1), skip_group_check=True)
                    nc.scalar.copy(out=stats[:, dst_sl], in_=p_ps)

                col_sum(slice(0, O), lambda m: sumu[:, m, :])                 # P1
                vs_tiles = []
                for m in range(T):
                    vs_t = sb.tile([P, O], f32, tag="vst", bufs=2,
                                   name=f"vs{i}_{m}")
                    nc.vector.tensor_mul(vs_t, vrm[:, m, :], sumu[:, m, :])
                    vs_tiles.append(vs_t)
                col_sum(slice(2 * O, 3 * O), lambda m: vs_tiles[m])           # P3
                col_sum(slice(3 * O, 4 * O), lambda m: vrm[:, m, :])          # P4
                v2_tiles = []
                for m in range(T):
                    v2_t = sb.tile([P, O], f32, tag="v2t", bufs=2,
                                   name=f"v2{i}_{m}")
                    nc.scalar.square(out=v2_t, in_=vrm[:, m, :])
                    v2_tiles.append(v2_t)
                col_sum(slice(4 * O, 5 * O), lambda m: v2_tiles[m])           # P5
                # P2 = sum_j cnt_j * U_j^2
                cnt_sb = sb.tile([1, n], f32, tag="cntsb", name=f"csb{i}")
                for ci in range(CH):
                    nc.scalar.copy(out=cnt_sb[:, ci * 512:(ci + 1) * 512],
                                   in_=cnt_ps[ci])
                cnt_t = sb.tile([P, T], f32, tag="cntt", name=f"ctt{i}")
                for m in range(T):
                    tp = ps.tile([P, 1], f32, tag="aux", name=f"ctp{i}_{m}")
                    nc.tensor.matmul(out=tp, lhsT=cnt_sb[:, m * P:(m + 1) * P],
                                     rhs=ident[0:1, 0:1], start=True, stop=True,
                                     is_transpose=True)
                    nc.scalar.copy(out=cnt_t[:, m:m + 1], in_=tp)
                u2_tiles = []
                for m in range(T):
                    u2_t = sb.tile([P, O], f32, tag="u2t", bufs=2,
                                   name=f"u2{i}_{m}")
                    nc.scalar.square(out=u2_t, in_=urm[:, m, :])
                    u2_tiles.append(u2_t)
                col_sum(slice(O, 2 * O), lambda m: u2_tiles[m],
                        lhs=lambda m: cnt_t[:, m:m + 1])                      # P2

                # ---- AllReduce of the 5 stat vectors ----
                st_in = dr.tile([1, 5 * OMAX], f32, tag="stin", name=f"sti{i}")
                st_out = dr.tile([1, 5 * OMAX], f32, tag="stout", name=f"sto{i}")
                nc.gpsimd.dma_start(st_in[:, 0:5 * O], stats)
                nc.gpsimd.collective_compute(
                    "AllReduce", mybir.AluOpType.add,
                    replica_groups=[list(range(num_cores))],
                    ins=[st_in[:, 0:5 * O].opt()],
                    outs=[st_out[:, 0:5 * O].opt()])
                allr = sb.tile([1, 5 * O], f32, tag="allr", name=f"ar{i}")
                nc.gpsimd.dma_start(allr, st_out[:, 0:5 * O])

                # ---- finalize BN scale/shift ----
                p1, p2 = allr[:, 0:O], allr[:, O:2 * O]
                p3, p4, p5 = (allr[:, 2 * O:3 * O], allr[:, 3 * O:4 * O],
                              allr[:, 4 * O:5 * O])
                fin = sb.tile([1, 6 * O], f32, tag="fin", name=f"fin{i}")
                mean, e2 = fin[:, 0:O], fin[:, O:2 * O]
                var, s_v, t_v, tmp = (fin[:, 2 * O:3 * O], fin[:, 3 * O:4 * O],
                                      fin[:, 4 * O:5 * O], fin[:, 5 * O:6 * O])
                inv_m = 1.0 / float(M_total)
                nc.vector.scalar_tensor_tensor(out=mean, in0=p4,
                                               scalar=float(KNN), in1=p1,
                                               op0=mybir.AluOpType.mult,
                                               op1=mybir.AluOpType.add)
                nc.vector.tensor_scalar_mul(mean, mean, inv_m)
                nc.vector.scalar_tensor_tensor(out=e2, in0=p3, scalar=2.0,
                                               in1=p2,
                                               op0=mybir.AluOpType.mult,
                                               op1=mybir.AluOpType.add)
                nc.vector.scalar_tensor_tensor(out=e2, in0=p5,
                                               scalar=float(KNN), in1=e2,
                                               op0=mybir.AluOpType.mult,
                                               op1=mybir.AluOpType.add)
                nc.vector.tensor_scalar_mul(e2, e2, inv_m)
                nc.vector.tensor_mul(var, mean, mean)
                nc.vector.tensor_sub(var, e2, var)
                nc.vector.tensor_scalar_add(var, var, EPS)
                nc.vector.reciprocal(tmp, var)
                nc.scalar.sqrt(out=tmp, in_=tmp)
                nc.vector.tensor_mul(s_v, g_sb[i], tmp)
                nc.vector.tensor_mul(tmp, mean, s_v)
                nc.vector.tensor_sub(t_v, b_sb[i], tmp)
                s_rep = sb.tile([P, O], f32, tag="srep", name=f"srp{i}")
                t_rep = sb.tile([P, O], f32, tag="trep", name=f"trp{i}")
                nc.gpsimd.partition_broadcast(s_rep, s_v, channels=P)
                nc.gpsimd.partition_broadcast(t_rep, t_v, channels=P)

                # ---- assembly: out = lrelu(s*(maxu+V)+t) into vrm ----
                for m in range(T):
                    z = vrm[:, m, :]
                    nc.vector.tensor_add(z, maxu[:, m, :], z)
                    nc.vector.tensor_mul(z, z, s_rep)
                    nc.vector.tensor_add(z, z, t_rep)
                    nc.vector.scalar_tensor_tensor(out=z, in0=z, scalar=0.2,
                                                   in1=z,
                                                   op0=mybir.AluOpType.mult,
                                                   op1=mybir.AluOpType.max)

                c0 = sum(o for _, o in BLOCKS[:i])
                nc.sync.dma_start(
                    out=out_dram.ap().rearrange("(t p) f -> p t f",
                                                p=P)[:, :, c0:c0 + O],
                    in_=vrm)

                # transpose to channel-major for next block
                if i + 1 < len(BLOCKS):
                    for m in range(T):
                        for oc in range(NH):
                            ow = min(P, O - oc * P)
                            tp = ps.tile([P, P], f32, tag="eps",
                                         name=f"tp{i}_{m}_{oc}")
                            nc.tensor.matmul(
                                out=tp[0:ow, 0:P],
                                lhsT=vrm[:, m, oc * P:oc * P + ow],
                                rhs=ident, start=True, stop=True,
                                is_transpose=True)
                            nc.scalar.copy(
                                out=xcm_next[oc * P:oc * P + ow,
                                             m * P:(m + 1) * P],
                                in_=tp[0:ow, 0:P])

    nc.compile()
    return nc


_CACHE = {}


def _get_nc(num_cores=NCORES, n=N):
    key = (num_cores, n)
    if key not in _CACHE:
        _CACHE[key] = build_nc(num_cores, n)
    return _CACHE[key]


def make_in_maps(x, weights, num_cores=NCORES):
    """x [B, n, 3]; weights = [(W, g, b), ...] with W [O, 2C]."""
    base = {}
    for i, ((W, g, b), (c, o)) in enumerate(zip(weights, BLOCKS)):
        W = np.asarray(W, np.float32)
        base[f"wn{i}"] = np.ascontiguousarray(W[:, :c].T)
        base[f"wd{i}"] = np.ascontiguousarray((W[:, c:] - W[:, :c]).T)
        base[f"g{i}"] = np.asarray(g, np.float32).reshape(1, o)
        base[f"b{i}"] = np.asarray(b, np.float32).reshape(1, o)
    base["ident_in"] = np.eye(P, dtype=np.float32)
    in_maps = []
    for ci in range(num_cores):
        m = dict(base)
        m["xt_in"] = np.ascontiguousarray(np.asarray(x[ci], np.float32).T)
        in_maps.append(m)
    return in_maps


def kernel(x, W1, g1, b1, W2, g2, b2, W3, g3, b3, W4, g4, b4, k):
    from concourse import bass_utils
    assert int(k) == KNN
    x = np.asarray(x, np.float32)
    assert x.shape == (NCORES, N, 3)
    weights = [(W1, g1, b1), (W2, g2, b2), (W3, g3, b3), (W4, g4, b4)]
    nc = _get_nc()
    in_maps = make_in_maps(x, weights)
    res = bass_utils.run_bass_kernel_spmd(nc, in_maps,
                                          core_ids=list(range(NCORES)))
    return np.stack([res.results[c]["out"] for c in range(NCORES)], axis=0)


# revision 5
# speedup vs baseline: 1.2678x; 1.2678x over previous
"""DGCNN (4x EdgeConv) Trainium2 Bass kernel.

Sharding: data-parallel over batch B=8 across 8 NeuronCores (one point cloud
per core). BN batch statistics are combined with a tiny (5*O floats) AllReduce
per block.

Key algebraic restructuring: with W = [Wn | Wc] the edge conv
    h[n,kk,:] = (x[idx[n,kk]] - x[n]) @ Wn.T + x[n] @ Wc.T
              = U[idx[n,kk]] + V[n],   U = X @ Wn.T,  V = X @ (Wc-Wn).T
so the per-edge matmul (N*k rows) becomes two per-point matmuls (20x fewer
FLOPs) followed by a row gather of U (dma_gather). Since BN scale s>0 and
LeakyReLU is monotone increasing, max over k commutes with the activation:
    out[n] = lrelu(s * (max_k U[idx[n,k]] + V[n]) + t)
BN statistics over h are recovered exactly from reductions:
    sum(h)   = sum(sumU) + k*sum(V)
    sum(h^2) = sum_j cnt_j U_j^2 + 2*sum(V*sumU) + k*sum(V^2)
where cnt_j is the in-degree of point j (column sums of the kNN selection
mask, done on the tensor engine) and sumU[n] = sum_k U[idx[n,k]].

kNN: E = 2*X^T X - sq_j (row-shifted negative squared distance; the -sq_i
term is constant per row and irrelevant for row-wise top-k) via an augmented
fp32 matmul; top-20 per row via 3 rounds of max8/max_index/match_replace.
"""

import numpy as np

N = 2048
KNN = 20
NCORES = 8
P = 128
BLOCKS = [(3, 64), (64, 64), (64, 128), (128, 256)]  # (C_in, C_out)
OUT_F = sum(o for _, o in BLOCKS)  # 512
EPS = 1e-5
NEG = -1e30


def build_nc(num_cores=NCORES, n=N):
    import concourse.bacc as bacc
    import concourse.mybir as mybir
    import concourse.tile as tile
    from concourse import library_config

    f32 = mybir.dt.float32
    bf16 = mybir.dt.bfloat16
    T = n // P          # point tiles per cloud
    CH = n // 512       # 512-wide chunks per row
    M_total = num_cores * n * KNN

    nc = bacc.Bacc("TRN2", target_bir_lowering=False, debug=False,
                   num_devices=num_cores)

    xt_in = nc.dram_tensor("xt_in", [3, n], f32, kind="ExternalInput")
    ident_in = nc.dram_tensor("ident_in", [P, P], f32, kind="ExternalInput")
    wn_in, wd_in, g_in, b_in = {}, {}, {}, {}
    for i, (c, o) in enumerate(BLOCKS):
        wn_in[i] = nc.dram_tensor(f"wn{i}", [c, o], f32, kind="ExternalInput")
        wd_in[i] = nc.dram_tensor(f"wd{i}", [c, o], f32, kind="ExternalInput")
        g_in[i] = nc.dram_tensor(f"g{i}", [1, o], f32, kind="ExternalInput")
        b_in[i] = nc.dram_tensor(f"b{i}", [1, o], f32, kind="ExternalInput")
    out_dram = nc.dram_tensor("out", [n, OUT_F], f32, kind="ExternalOutput")

    OMAX = max(o for _, o in BLOCKS)

    with tile.TileContext(nc) as tc:
        with (
            tc.tile_pool(name="consts", bufs=1) as cp,
            tc.tile_pool(name="sb", bufs=1) as sb,
            tc.tile_pool(name="ps", bufs=2, space="PSUM") as ps,
            tc.tile_pool(name="dr", bufs=1, space="DRAM") as dr,
        ):
            nc.gpsimd.load_library(library_config.mlp)

            ident = cp.tile([P, P], f32)
            nc.sync.dma_start(out=ident, in_=ident_in.ap())
            ones_f = cp.tile([P, 1], f32)
            nc.vector.memset(ones_f, 1.0)
            ones_bf = cp.tile([P, 1], bf16)
            nc.vector.memset(ones_bf, 1.0)
            negones = cp.tile([1, P], f32)
            nc.vector.memset(negones, -1.0)
            wn_sb, wd_sb, g_sb, b_sb = {}, {}, {}, {}
            for i, (c, o) in enumerate(BLOCKS):
                wn_sb[i] = cp.tile([c, o], f32, name=f"wn_sb{i}")
                nc.sync.dma_start(out=wn_sb[i], in_=wn_in[i].ap())
                wd_sb[i] = cp.tile([c, o], f32, name=f"wd_sb{i}")
                nc.sync.dma_start(out=wd_sb[i], in_=wd_in[i].ap())
                g_sb[i] = cp.tile([1, o], f32, name=f"g_sb{i}")
                nc.sync.dma_start(out=g_sb[i], in_=g_in[i].ap())
                b_sb[i] = cp.tile([1, o], f32, name=f"b_sb{i}")
                nc.sync.dma_start(out=b_sb[i], in_=b_in[i].ap())

            xcm_tiles = [sb.tile([P, n], f32, name=f"xcm{j}", tag=f"xcm{j}")
                         for j in range(2)]
            nc.sync.dma_start(out=xcm_tiles[0][0:3, :], in_=xt_in.ap())

            for i, (C, O) in enumerate(BLOCKS):
                xcm = xcm_tiles[i % 2][0:C, :]
                xcm_next = xcm_tiles[(i + 1) % 2]
                wn, wd = wn_sb[i], wd_sb[i]
                OH = min(O, P)          # channel half for gather of O=256
                NH = 2 if O > P else 1

                # ---- prep: sq, 2x, U, V ----
                xtmp = sb.tile([P, n], f32, tag="xtmp", name=f"xtmp{i}")
                nc.scalar.square(out=xtmp[0:C, :], in_=xcm)
                sq_row = sb.tile([1, n], f32, tag="sq", name=f"sqr{i}")
                for ci in range(CH):
                    sl = slice(ci * 512, (ci + 1) * 512)
                    sq_ps = ps.tile([1, 512], f32, tag="aux", name=f"sqps{i}_{ci}")
                    nc.tensor.matmul(out=sq_ps, lhsT=ones_f[0:C, :],
                                     rhs=xtmp[0:C, sl], start=True, stop=True)
                    nc.scalar.copy(out=sq_row[:, sl], in_=sq_ps)
                xcm2 = sb.tile([P, n], f32, tag="xcm2", name=f"xcm2_{i}")
                nc.vector.tensor_scalar_mul(xcm2[0:C, :], xcm, 2.0)

                urm = sb.tile([P, T, O], f32, tag="urm", name=f"urm{i}")
                vrm = sb.tile([P, T, O], f32, tag="vrm", name=f"vrm{i}")
                for m in range(T):
                    msl = slice(m * P, (m + 1) * P)
                    for di, (dst, w_) in enumerate(((urm, wn), (vrm, wd))):
                        ups = ps.tile([P, O], f32, tag="aux",
                                      name=f"uv{i}_{m}_{di}")
                        nc.tensor.matmul(out=ups, lhsT=xcm[:, msl], rhs=w_,
                                         start=True, stop=True)
                        nc.scalar.copy(out=dst[:, m, :], in_=ups)
                u_dram = dr.tile([n, O], f32, tag="udram", name=f"ud{i}")
                nc.sync.dma_start(
                    out=u_dram.rearrange("(t p) o -> p t o", p=P), in_=urm)

                # ---- kNN + gather + windowed reduces ----
                idx_all = sb.tile([P, T, KNN], mybir.dt.int16, tag="idxall",
                                  name=f"idxall{i}")
                maxu = sb.tile([P, T, O], f32, tag="maxu", name=f"maxu{i}")
                sumu = sb.tile([P, T, O], f32, tag="sumu", name=f"sumu{i}")
                cnt_ps = [ps.tile([1, 512], f32, tag=f"cnt{ci}", bufs=1,
                                  name=f"cnt{i}_{ci}") for ci in range(CH)]

                def do_gather_tile(m, w_grp, ti, i=i, u_dram=u_dram, maxu=maxu,
                                   sumu=sumu, O=O, OH=OH, NH=NH):
                    for h in range(NH):
                        g_t = sb.tile([P, KNN, OH], f32, tag="gt", bufs=2,
                                      name=f"g{i}_{m}_{h}")
                        nc.gpsimd.dma_gather(
                            out_ap=g_t,
                            in_ap=u_dram[:, h * OH:(h + 1) * OH],
                            idxs_ap=w_grp[:, ti * 160:(ti + 1) * 160],
                            num_idxs=P * KNN, num_idxs_reg=P * KNN,
                            elem_size=OH,
                            elem_step=O if NH > 1 else None,
                            single_packet=False)
                        gv = g_t.rearrange("p k o -> p o k")
                        osl = slice(h * OH, (h + 1) * OH)
                        nc.vector.tensor_reduce(
                            out=maxu[:, m, osl], in_=gv,
                            axis=mybir.AxisListType.X, op=mybir.AluOpType.max)
                        nc.vector.tensor_reduce(
                            out=sumu[:, m, osl], in_=gv,
                            axis=mybir.AxisListType.X, op=mybir.AluOpType.add)

                GRP = 4
                for m in range(T):
                    msl = slice(m * P, (m + 1) * P)
                    e_m = sb.tile([P, n], f32, tag="em", bufs=2, name=f"e{i}_{m}")
                    for ci in range(CH):
                        sl = slice(ci * 512, (ci + 1) * 512)
                        eps_t = ps.tile([P, 512], f32, tag="eps",
                                        name=f"eps{i}_{m}_{ci}")
                        nc.tensor.matmul(out=eps_t, lhsT=xcm2[0:C, msl],
                                         rhs=xcm[:, sl], start=True, stop=False,
                                         skip_group_check=True)
                        nc.tensor.matmul(out=eps_t, lhsT=negones,
                                         rhs=sq_row[:, sl], start=False,
                                         stop=True, skip_group_check=True)
                        nc.scalar.copy(out=e_m[:, sl], in_=eps_t)

                    vals = sb.tile([P, 24], f32, tag="vals", bufs=2,
                                   name=f"v{i}_{m}")
                    idxu = sb.tile([P, 24], mybir.dt.uint32, tag="idxu", bufs=2,
                                   name=f"ixu{i}_{m}")
                    e_w = sb.tile([P, n], f32, tag="ew", bufs=1, name=f"ew{i}_{m}")
                    for r in range(3):
                        src = e_m if r == 0 else e_w
                        v8 = vals[:, r * 8:(r + 1) * 8]
                        nc.vector.max(out=v8, in_=src)
                        nc.vector.max_index(out=idxu[:, r * 8:(r + 1) * 8],
                                            in_max=v8, in_values=e_m)
                        if r < 2:
                            nc.vector.match_replace(out=e_w, in_to_replace=v8,
                                                    in_values=src, imm_value=NEG)
                    # selection mask + in-degree partial sums (PE column sums)
                    s_m = sb.tile([P, n], bf16, tag="sm", bufs=2, name=f"s{i}_{m}")
                    nc.vector.tensor_scalar(out=s_m, in0=e_m,
                                            scalar1=vals[:, 19:20], scalar2=None,
                                            op0=mybir.AluOpType.is_ge)
                    for ci in range(CH):
                        sl = slice(ci * 512, (ci + 1) * 512)
                        nc.tensor.matmul(out=cnt_ps[ci], lhsT=ones_bf,
                                         rhs=s_m[:, sl], start=(m == 0),
                                         stop=(m == T - 1), skip_group_check=True)
                    nc.vector.tensor_copy(idx_all[:, m, :], idxu[:, 0:KNN])

                    if m % GRP == GRP - 1:
                        g0 = m - (GRP - 1)
                        w_grp = sb.tile([P, GRP * 160], mybir.dt.int16,
                                        tag="wgrp", bufs=2, name=f"w{i}_{m}")
                        wv = w_grp.rearrange("l (t c pg) -> l t c pg", t=GRP, pg=8)
                        for pg in range(8):
                            nc.sync.dma_start(
                                out=wv[0:16, :, :, pg],
                                in_=idx_all[pg * 16:(pg + 1) * 16,
                                            g0:g0 + GRP, 0:KNN])
                        for rep in range(1, 8):
                            nc.scalar.dma_start(
                                out=w_grp[rep * 16:(rep + 1) * 16, :],
                                in_=w_grp[0:16, :])
                        for ti in range(GRP):
                            do_gather_tile(g0 + ti, w_grp, ti)

                # ---- BN statistics (per-channel column sums on PE) ----
                stats = sb.tile([1, 5 * O], f32, tag="stats", name=f"stt{i}")

                def col_sum(dst_sl, rhs_fn, lhs=None, i=i, stats=stats, O=O):
                    p_ps = ps.tile([1, O], f32, tag="aux",
                                   name=f"pps{i}_{dst_sl.start}")
                    for m in range(T):
                        nc.tensor.matmul(out=p_ps,
                                         lhsT=(ones_f if lhs is None else lhs(m)),
                                         rhs=rhs_fn(m), start=(m == 0),
                                         stop=(m == T - 1), skip_group_check=True)
                    nc.scalar.copy(out=stats[:, dst_sl], in_=p_ps)

                col_sum(slice(0, O), lambda m: sumu[:, m, :])                 # P1
                vs_tiles = []
                for m in range(T):
                    vs_t = sb.tile([P, O], f32, tag="vst", bufs=2,
                                   name=f"vs{i}_{m}")
                    nc.vector.tensor_mul(vs_t, vrm[:, m, :], sumu[:, m, :])
                    vs_tiles.append(vs_t)
                col_sum(slice(2 * O, 3 * O), lambda m: vs_tiles[m])           # P3
                col_sum(slice(3 * O, 4 * O), lambda m: vrm[:, m, :])          # P4
                v2_tiles = []
                for m in range(T):
                    v2_t = sb.tile([P, O], f32, tag="v2t", bufs=2,
                                   name=f"v2{i}_{m}")
                    nc.scalar.square(out=v2_t, in_=vrm[:, m, :])
                    v2_tiles.append(v2_t)
                col_sum(slice(4 * O, 5 * O), lambda m: v2_tiles[m])           # P5
                # P2 = sum_j cnt_j * U_j^2
                cnt_sb = sb.tile([1, n], f32, tag="cntsb", name=f"csb{i}")
                for ci in range(CH):
                    nc.scalar.copy(out=cnt_sb[:, ci * 512:(ci + 1) * 512],
                                   in_=cnt_ps[ci])
                cnt_t = sb.tile([P, T], f32, tag="cntt", name=f"ctt{i}")
                for m in range(T):
                    tp = ps.tile([P, 1], f32, tag="aux", name=f"ctp{i}_{m}")
                    nc.tensor.matmul(out=tp, lhsT=cnt_sb[:, m * P:(m + 1) * P],
                                     rhs=ident[0:1, 0:1], start=True, stop=True,
                                     is_transpose=True)
                    nc.scalar.copy(out=cnt_t[:, m:m + 1], in_=tp)
                u2_tiles = []
                for m in range(T):
                    u2_t = sb.tile([P, O], f32, tag="u2t", bufs=2,
                                   name=f"u2{i}_{m}")
                    nc.scalar.square(out=u2_t, in_=urm[:, m, :])
                    u2_tiles.append(u2_t)
                col_sum(slice(O, 2 * O), lambda m: u2_tiles[m],
                        lhs=lambda m: cnt_t[:, m:m + 1])                      # P2

                # ---- AllReduce of the 5 stat vectors ----
                st_in = dr.tile([1, 5 * OMAX], f32, tag="stin", name=f"sti{i}")
                st_out = dr.tile([1, 5 * OMAX], f32, tag="stout", name=f"sto{i}")
                nc.gpsimd.dma_start(st_in[:, 0:5 * O], stats)
                nc.gpsimd.collective_compute(
                    "AllReduce", mybir.AluOpType.add,
                    replica_groups=[list(range(num_cores))],
                    ins=[st_in[:, 0:5 * O].opt()],
                    outs=[st_out[:, 0:5 * O].opt()])
                allr = sb.tile([1, 5 * O], f32, tag="allr", name=f"ar{i}")
                nc.gpsimd.dma_start(allr, st_out[:, 0:5 * O])

                # ---- finalize BN scale/shift ----
                p1, p2 = allr[:, 0:O], allr[:, O:2 * O]
                p3, p4, p5 = (allr[:, 2 * O:3 * O], allr[:, 3 * O:4 * O],
                              allr[:, 4 * O:5 * O])
                fin = sb.tile([1, 6 * O], f32, tag="fin", name=f"fin{i}")
                mean, e2 = fin[:, 0:O], fin[:, O:2 * O]
                var, s_v, t_v, tmp = (fin[:, 2 * O:3 * O], fin[:, 3 * O:4 * O],
                                      fin[:, 4 * O:5 * O], fin[:, 5 * O:6 * O])
                inv_m = 1.0 / float(M_total)
                nc.vector.scalar_tensor_tensor(out=mean, in0=p4,
                                               scalar=float(KNN), in1=p1,
                                               op0=mybir.AluOpType.mult,
                                               op1=mybir.AluOpType.add)
                nc.vector.tensor_scalar_mul(mean, mean, inv_m)
                nc.vector.scalar_tensor_tensor(out=e2, in0=p3, scalar=2.0,
                                               in1=p2,
                                               op0=mybir.AluOpType.mult,
                                               op1=mybir.AluOpType.add)
                nc.vector.scalar_tensor_tensor(out=e2, in0=p5,
                                               scalar=float(KNN), in1=e2,
                                               op0=mybir.AluOpType.mult,
                                               op1=mybir.AluOpType.add)
                nc.vector.tensor_scalar_mul(e2, e2, inv_m)
                nc.vector.tensor_mul(var, mean, mean)
                nc.vector.tensor_sub(var, e2, var)
                nc.vector.tensor_scalar_add(var, var, EPS)
                nc.vector.reciprocal(tmp, var)
                nc.scalar.sqrt(out=tmp, in_=tmp)
                nc.vector.tensor_mul(s_v, g_sb[i], tmp)
                nc.vector.tensor_mul(tmp, mean, s_v)
                nc.vector.tensor_sub(t_v, b_sb[i], tmp)
                s_rep = sb.tile([P, O], f32, tag="srep", name=f"srp{i}")
                t_rep = sb.tile([P, O], f32, tag="trep", name=f"trp{i}")
                nc.gpsimd.partition_broadcast(s_rep, s_v, channels=P)
                nc.gpsimd.partition_broadcast(t_rep, t_v, channels=P)

                # ---- assembly: out = lrelu(s*(maxu+V)+t) into vrm ----
                for m in range(T):
                    z = vrm[:, m, :]
                    nc.vector.tensor_add(z, maxu[:, m, :], z)
                    nc.vector.tensor_mul(z, z, s_rep)
                    nc.vector.tensor_add(z, z, t_rep)
                    nc.vector.scalar_tensor_tensor(out=z, in0=z, scalar=0.2,
                                                   in1=z,
                                                   op0=mybir.AluOpType.mult,
                                                   op1=mybir.AluOpType.max)

                c0 = sum(o for _, o in BLOCKS[:i])
                nc.sync.dma_start(
                    out=out_dram.ap().rearrange("(t p) f -> p t f",
                                                p=P)[:, :, c0:c0 + O],
                    in_=vrm)

                # transpose to channel-major for next block
                if i + 1 < len(BLOCKS):
                    for m in range(T):
                        for oc in range(NH):
                            ow = min(P, O - oc * P)
                            tp = ps.tile([P, P], f32, tag="eps",
                                         name=f"tp{i}_{m}_{oc}")
                            nc.tensor.matmul(
                                out=tp[0:ow, 0:P],
                                lhsT=vrm[:, m, oc * P:oc * P + ow],
                                rhs=ident, start=True, stop=True,
                                is_transpose=True)
                            nc.scalar.copy(
                                out=xcm_next[oc * P:oc * P + ow,
                                             m * P:(m + 1) * P],
                                in_=tp[0:ow, 0:P])

    nc.compile()
    return nc


_CACHE = {}


def _get_nc(num_cores=NCORES, n=N):
    key = (num_cores, n)
    if key not in _CACHE:
        _CACHE[key] = build_nc(num_cores, n)
    return _CACHE[key]


def make_in_maps(x, weights, num_cores=NCORES):
    """x [B, n, 3]; weights = [(W, g, b), ...] with W [O, 2C]."""
    base = {}
    for i, ((W, g, b), (c, o)) in enumerate(zip(weights, BLOCKS)):
        W = np.asarray(W, np.float32)
        base[f"wn{i}"] = np.ascontiguousarray(W[:, :c].T)
        base[f"wd{i}"] = np.ascontiguousarray((W[:, c:] - W[:, :c]).T)
        base[f"g{i}"] = np.asarray(g, np.float32).reshape(1, o)
        base[f"b{i}"] = np.asarray(b, np.float32).reshape(1, o)
    base["ident_in"] = np.eye(P, dtype=np.float32)
    in_maps = []
    for ci in range(num_cores):
        m = dict(base)
        m["xt_in"] = np.ascontiguousarray(np.asarray(x[ci], np.float32).T)
        in_maps.append(m)
    return in_maps


class _Runner:
    """Cached PJRT executable for the 8-core SPMD NEFF (mirrors
    bass2jax.run_bass_via_pjrt's multi-core branch, but jits once)."""

    def __init__(self, nc, n_cores):
        import jax
        import jax.numpy as jnp
        from jax.sharding import Mesh, PartitionSpec
        from concourse import bass2jax
        import concourse.mybir as mybir
        try:
            from jax.experimental.shard_map import shard_map
        except ImportError:
            from jax import shard_map

        bass2jax.install_neuronx_cc_hook()
        self.nc = nc
        self.n_cores = n_cores
        in_names, out_names, out_avals, zero_outs = [], [], [], []
        for alloc in nc.m.functions[0].allocations:
            if not isinstance(alloc, mybir.MemoryLocationSet):
                continue
            name = alloc.memorylocations[0].name
            if alloc.kind == "ExternalInput":
                if nc.partition_id_tensor is None or name != nc.partition_id_tensor.name:
                    in_names.append(name)
            elif alloc.kind == "ExternalOutput":
                shape = tuple(alloc.tensor_shape)
                dtype = mybir.dt.np(alloc.dtype)
                out_names.append(name)
                out_avals.append(jax.core.ShapedArray(shape, dtype))
                zero_outs.append(np.zeros(shape, dtype))
        self.in_names, self.out_names = in_names, out_names
        self.out_avals, self.zero_outs = out_avals, zero_outs
        n_params, n_outs = len(in_names), len(out_names)
        all_in_names = list(in_names) + list(out_names)
        if nc.partition_id_tensor is not None:
            all_in_names.append(nc.partition_id_tensor.name)

        def _body(*args):
            operands = list(args)
            if nc.partition_id_tensor is not None:
                operands.append(bass2jax.partition_id_tensor())
            outs = bass2jax._bass_exec_p.bind(
                *operands,
                out_avals=tuple(out_avals),
                in_names=tuple(all_in_names),
                out_names=tuple(out_names),
                lowering_input_output_aliases=(),
                sim_require_finite=True,
                sim_require_nnan=True,
                nc=nc,
            )
            return tuple(outs)

        devices = jax.devices()[:n_cores]
        mesh = Mesh(np.asarray(devices), ("core",))
        in_specs = (PartitionSpec("core"),) * (n_params + n_outs)
        out_specs = (PartitionSpec("core"),) * n_outs
        self.fn = jax.jit(
            shard_map(_body, mesh=mesh, in_specs=in_specs, out_specs=out_specs,
                      check_rep=False),
            donate_argnums=tuple(range(n_params, n_params + n_outs)),
            keep_unused=True)

    def __call__(self, in_maps):
        concat_in = [
            np.concatenate([np.asarray(in_maps[c][name])
                            for c in range(self.n_cores)], axis=0)
            for name in self.in_names
        ]
        concat_zeros = [
            np.zeros((self.n_cores * z.shape[0], *z.shape[1:]), z.dtype)
            for z in self.zero_outs
        ]
        out_arrs = self.fn(*concat_in, *concat_zeros)
        return [
            {name: np.asarray(out_arrs[i]).reshape(
                self.n_cores, *self.out_avals[i].shape)[c]
             for i, name in enumerate(self.out_names)}
            for c in range(self.n_cores)
        ]


_RUNNER = [None]


def kernel(x, W1, g1, b1, W2, g2, b2, W3, g3, b3, W4, g4, b4, k):
    assert int(k) == KNN
    x = np.asarray(x, np.float32)
    assert x.shape == (NCORES, N, 3)
    weights = [(W1, g1, b1), (W2, g2, b2), (W3, g3, b3), (W4, g4, b4)]
    if _RUNNER[0] is None:
        _RUNNER[0] = _Runner(_get_nc(), NCORES)
    in_maps = make_in_maps(x, weights)
    res = _RUNNER[0](in_maps)
    return np.stack([res[c]["out"] for c in range(NCORES)], axis=0)
